# revision 19
# baseline (speedup 1.0000x reference)
"""Trainium2 Bass kernel for nn_MeshTransformer (8-core SPMD, V-sharded).

Computes, for each of BS=256 (b,s) pairs:
    out[bs, v, i] = sum_{p,j} ws[bs,p] * R[i,j](bs,p) * deformed[p,v,j]
                    + sum_p w[bs,p] * t[bs,p,i]
with R the XYZ-euler rotation, ws = w * scale, deformed = base + offsets.

Mapping:
  - Vertex dim V (2562, padded to 2576) is sharded 8 ways (322/core).
  - Each core computes all 256 weight matrices on-chip and contracts them
    against its deformed slice on the PE (fp16 matmuls, fp32 PSUM).
  - The host ships six 256-col angle blocks, each range-folded to [-pi, pi)
    (Sin spline domain) and pre-shifted so that ONE ACT Sin op yields every
    needed trig operand, including the stacked/negated forms, as views:
      S = sin(ang6) = [ sa | ca | (sc;cc) | (cc;sc) | (sb;-sb) | (cb;-cb) ]
    (cos(x) = sin(pi/2 - x); the two 64-partition halves of a block hold
    different shifts, matching the lhsT partition packing below.)
  - lhsT partition layout packs rotation column j in 64-partition blocks,
    paired with a stacked rhs:
      LA_i = [W_i0 (p 0..63) ; W_i1 (p 64..127)]   DA = [deformed_0 ; deformed_1]
      LB_i = [W_i2          ; wt_i            ]   DB = [deformed_2 ; ones     ]
    (the ones block folds the translation term into the same contraction),
    so each rotation-row build is a single full-lane DVE op:
      LA0 = WS*(CBX*UX), LA1 = WCA*U + WSA*V, LA2 = WSA*U - WCA*V, V = SBX*UX
  - PSUM is drained to fp16 plane tiles and DMA'd as 6 [128, VC] planes;
    the host gather transposes to the reference [BS, V, 3] layout.
"""

import numpy as np
from contextlib import ExitStack

import concourse.bass as bass
import concourse.tile as tile
from concourse import bacc, mybir
from concourse.bass_utils import run_bass_kernel_spmd

B, S, P, V = 16, 16, 64, 2562
BS = B * S              # 256
N_CORES = 8
VPAD = 2576             # multiple of 8; per-core N kept even
VC = VPAD // N_CORES    # 322 vertices per core

F32 = mybir.dt.float32
F16 = mybir.dt.float16
AF = mybir.ActivationFunctionType
ALU = mybir.AluOpType


def _build_kernel():
    nc = bacc.Bacc("TRN2", target_bir_lowering=False, debug=False)

    ang_d = nc.dram_tensor("ang6", [128, 1536], F16, kind="ExternalInput").ap()
    wst_d = nc.dram_tensor("wst", [128, 1280], F16, kind="ExternalInput").ap()
    # offtA | bsetA | offtB/bsetB (rows 0:64)
    dmat_d = nc.dram_tensor("dmat", [128, 4 * VC], F16, kind="ExternalInput").ap()
    out_d = nc.dram_tensor("out", [6, 128, VC], F16, kind="ExternalOutput").ap()

    lo = slice(0, 64)
    hi = slice(64, 128)

    with tile.TileContext(nc) as tc, ExitStack() as ctx:
        pool = ctx.enter_context(tc.tile_pool(name="work", bufs=1))
        psum = ctx.enter_context(tc.tile_pool(name="psum", bufs=6, space="PSUM"))

        # preload the ACT Sin table set while the inputs are still in flight
        dummy = pool.tile([128, 1], F16, tag="dummy")
        dummy2 = pool.tile([128, 1], F16, tag="dummy2")
        nc.vector.memset(dummy[:], 0.25)
        nc.scalar.activation(dummy2[:], dummy[:], AF.Sin)

        # ---- input tiles ----
        ang = pool.tile([128, 1536], F16, tag="ang")
        wst = pool.tile([128, 1280], F16, tag="wst")
        dmat = pool.tile([128, 4 * VC], F16, tag="dmat")
        nc.sync.dma_start(out=ang[:], in_=ang_d[:])
        nc.sync.dma_start(out=wst[:], in_=wst_d[:])
        nc.sync.dma_start(out=dmat[:], in_=dmat_d[:])
        wraw = wst[:, 0:BS]
        scl = wst[:, BS:2 * BS]
        trn = wst[:, 2 * BS:5 * BS]
        dta = dmat[:, 0:2 * VC]                  # offtA | bsetA
        dtb = dmat[0:64, 2 * VC:4 * VC]          # offtB | bsetB (rows 0:64)

        # ---- deformed (rhs) ----
        da = pool.tile([128, VC], F16, tag="da")
        db = pool.tile([128, VC], F16, tag="db")
        nc.vector.memset(db[64:128, :], 1.0)         # translation ones block
        nc.vector.tensor_add(da[:], dta[:, 0:VC], dta[:, VC:2 * VC])
        nc.vector.tensor_add(db[0:64, :], dtb[:, 0:VC], dtb[:, VC:2 * VC])

        # ---- trig: one Sin over all pre-folded blocks ----
        sall = pool.tile([128, 1536], F16, tag="sall")
        nc.scalar.activation(sall[:], ang[:], AF.Sin)
        sa = sall[:, 0:256]
        ca = sall[:, 256:512]
        u = sall[:, 512:768]        # [sc ; cc]
        ux = sall[:, 768:1024]      # [cc ; sc]
        sbx = sall[:, 1024:1280]    # [sb ; -sb]
        cbx = sall[:, 1280:1536]    # [cb ; -cb]

        # ---- weight products (fp16, full-lane) ----
        ws = pool.tile([128, BS], F16, tag="ws")
        wca = pool.tile([128, BS], F16, tag="wca")
        wsa = pool.tile([128, BS], F16, tag="wsa")
        p1 = pool.tile([128, BS], F16, tag="p1")      # [cbcc ; -cbsc]
        v = pool.tile([128, BS], F16, tag="v")        # [sbcc ; -sbsc]
        la = [pool.tile([128, BS], F16, name=f"la{i}", tag=f"la{i}") for i in range(3)]
        lb = [pool.tile([128, BS], F16, name=f"lb{i}", tag=f"lb{i}") for i in range(3)]
        ta = pool.tile([128, BS], F16, tag="ta")
        tb = pool.tile([128, BS], F16, tag="tb")
        tc_ = pool.tile([128, BS], F16, tag="tc_")
        td = pool.tile([128, BS], F16, tag="td")

        # translation weights: no trig dependency, run during the Sin op
        nc.gpsimd.tensor_mul(lb[0][hi, :], wraw[hi, :], trn[hi, 0:BS])
        nc.gpsimd.tensor_mul(lb[1][hi, :], wraw[hi, :], trn[hi, BS:2 * BS])
        nc.gpsimd.tensor_mul(lb[2][hi, :], wraw[hi, :], trn[hi, 2 * BS:3 * BS])

        nc.vector.tensor_mul(ws[:], wraw, scl)
        nc.vector.tensor_mul(wca[:], ws[:], ca)
        nc.vector.tensor_mul(wsa[:], ws[:], sa)

        # i=0 row first so PE can start early
        nc.vector.tensor_mul(p1[:], cbx, ux)
        nc.vector.tensor_mul(la[0][:], ws[:], p1[:])
        nc.vector.tensor_mul(v[:], sbx, ux)
        nc.vector.tensor_mul(lb[0][lo, :], ws[lo, :], sbx[lo, :])

        # i=1 row
        nc.vector.tensor_mul(ta[:], wca[:], u)
        nc.vector.tensor_mul(tb[:], wsa[:], v[:])
        nc.vector.tensor_add(la[1][:], ta[:], tb[:])
        nc.vector.scalar_tensor_tensor(
            lb[1][lo, :], cbx[lo, :], -1.0, wsa[lo, :], op0=ALU.mult, op1=ALU.mult)

        # i=2 row (td, lb2lo on gpsimd to shorten the DVE tail)
        nc.vector.tensor_mul(tc_[:], wsa[:], u)
        nc.gpsimd.tensor_mul(td[:], wca[:], v[:])
        nc.vector.tensor_sub(la[2][:], tc_[:], td[:])
        nc.gpsimd.tensor_mul(lb[2][lo, :], wca[lo, :], cbx[lo, :])

        # ---- matmuls + drain + output ----
        for i in range(3):
            for h in range(2):
                ms = slice(h * 128, (h + 1) * 128)
                ps = psum.tile([128, VC], F32)
                nc.tensor.matmul(ps[:], la[i][:, ms], da[:], start=True, stop=False)
                nc.tensor.matmul(ps[:], lb[i][:, ms], db[:], start=False, stop=True)
                osb = pool.tile([128, VC], F16, name=f"osb{i}{h}", tag=f"osb{i}{h}")
                if i == 2 and h == 0:
                    nc.vector.tensor_copy(osb[:], ps[:])
                else:
                    nc.scalar.copy(osb[:], ps[:])
                nc.sync.dma_start(out=out_d[i * 2 + h], in_=osb[:])

    nc.compile()
    return nc


_NC_CACHE = None


def _get_nc():
    global _NC_CACHE
    if _NC_CACHE is None:
        _NC_CACHE = _build_kernel()
    return _NC_CACHE


def _fold(x):
    """Range-fold to [-pi, pi) (Sin spline domain)."""
    return np.mod(x + np.pi, 2 * np.pi) - np.pi


def _prep_inputs(scales, transforms, prototype_weights, prototype_offsets, base_verts):
    """Host-side shard/layout prep (layout, dup, angle folding/shifting)."""
    f = np.float64
    hh = np.float16
    scl1 = np.asarray(scales, np.float32).reshape(BS)
    tf = np.asarray(transforms, np.float32).reshape(BS, P, 6)

    a = tf[:, :, 3].T.astype(f)   # [p, bs]
    b = tf[:, :, 4].T.astype(f)
    c = tf[:, :, 5].T.astype(f)
    P2 = np.pi / 2

    def blk(lov, hiv):
        return np.concatenate([_fold(lov), _fold(hiv)], axis=0)   # [128, bs]

    ang6 = np.concatenate([
        blk(a, a),              # sa
        blk(P2 - a, P2 - a),    # ca
        blk(c, P2 - c),         # [sc ; cc]
        blk(P2 - c, c),         # [cc ; sc]
        blk(b, -b),             # [sb ; -sb]
        blk(P2 - b, b - P2),    # [cb ; -cb]
    ], axis=1).astype(hh)                                         # [128, 1536]

    w_h = np.asarray(prototype_weights, np.float32).reshape(BS, P).T
    wraw = np.concatenate([w_h, w_h], axis=0)                     # [128, 256]
    scl = np.broadcast_to(scl1[None, :], (128, BS))
    trn_h = tf[:, :, 0:3].transpose(1, 2, 0).reshape(P, 3 * BS)
    trn = np.concatenate([trn_h, trn_h], axis=0)                  # [128, 768]

    offp = np.zeros((P, VPAD, 3), np.float32)
    offp[:, :V] = np.asarray(prototype_offsets, np.float32)
    offt = offp.transpose(2, 0, 1).reshape(192, VPAD)
    basep = np.zeros((VPAD, 3), np.float32)
    basep[:V] = np.asarray(base_verts, np.float32)
    bset = np.broadcast_to(basep.T[:, None, :], (3, P, VPAD)).reshape(192, VPAD)

    in_maps = []
    for core in range(N_CORES):
        vs = slice(core * VC, (core + 1) * VC)
        oA, bA = offt[0:128, vs], bset[0:128, vs]
        oB, bB = offt[128:192, vs], bset[128:192, vs]
        dB = np.zeros((128, 2 * VC), np.float32)
        dB[0:64, 0:VC] = oB
        dB[0:64, VC:2 * VC] = bB
        wst = np.concatenate([wraw, scl, trn], axis=1)
        dmat = np.concatenate([oA, bA, dB], axis=1)
        in_maps.append({"ang6": ang6, "wst": wst.astype(hh),
                        "dmat": dmat.astype(hh)})
    return in_maps


def kernel(scales, transforms, prototype_weights, prototype_offsets, base_verts):
    nc = _get_nc()
    in_maps = _prep_inputs(
        scales, transforms, prototype_weights, prototype_offsets, base_verts)
    res = run_bass_kernel_spmd(nc, in_maps, list(range(N_CORES)))
    full = np.empty((BS, VPAD, 3), np.float32)
    for c in range(N_CORES):
        planes = res.results[c]["out"].astype(np.float32)   # [6, 128, VC]
        vs = slice(c * VC, (c + 1) * VC)
        for i in range(3):
            for h in range(2):
                full[h * 128:(h + 1) * 128, vs, i] = planes[i * 2 + h]
    return np.ascontiguousarray(full[:, :V, :])
